# revision 1
# baseline (speedup 1.0000x reference)
"""Trainium2 Bass kernel for nn_NodeModel (GNN message passing), 8-core SPMD.

kernel(**inputs) takes full unsharded inputs, shards edges by destination-node
range across 8 cores, runs the Bass kernel, and reassembles the full output.
"""
import sys
sys.path.insert(0, '/opt/trn_rl_repo')
import numpy as np

import kprep
import kbuild

_cache = {}


def _prep(inputs):
    x = np.asarray(inputs["x"], np.float32)
    edge_index = np.asarray(inputs["edge_index"])
    edge_attr = np.asarray(inputs["edge_attr"], np.float32)
    u = np.asarray(inputs["u"], np.float32)
    batch = np.asarray(inputs["batch"])

    sched = kprep.build_schedule(edge_index[0], edge_index[1])
    arrs = kprep.build_arrays(sched, x, edge_index, edge_attr, u, batch)

    gb = np.stack([np.asarray(inputs[k], np.float32) for k in
                   ("g1a", "b1a", "g1b", "b1b", "g2a", "b2a", "g2b", "b2b")],
                  axis=1)
    comb = np.vstack([np.eye(64, dtype=np.float32)] * 2)
    ident = np.eye(64, dtype=np.float32)
    W2a = np.asarray(inputs["W2a"], np.float32)

    in_maps = []
    for c in range(8):
        npadseg = arrs["npadseg"][c]
        npadall = arrs["npadall"][c]
        npads = np.zeros((128, 2), np.float32)
        npads[0:64, 0] = npadseg[0]; npads[64:128, 0] = npadseg[1]
        npads[0:64, 1] = npadall[0]; npads[64:128, 1] = npadall[1]
        in_maps.append(dict(
            f=arrs["F"][c], xn=arrs["xn"][c], un=arrs["un"][c],
            invc=arrs["invc"][c], he=arrs["he"][c], mask=arrs["mask"][c],
            w1a=np.asarray(inputs["W1a"], np.float32),
            w1b=np.asarray(inputs["W1b"], np.float32),
            w2a1=np.ascontiguousarray(W2a[0:128]),
            w2a2=np.ascontiguousarray(W2a[128:192]),
            w2b=np.asarray(inputs["W2b"], np.float32),
            gb=gb, npads=npads, comb=comb, ident=ident,
        ))
    return sched, arrs, in_maps


def _get_runner(sched, repeat=1, R=60):
    key = (sched["T"], sched["S"], repeat, R,
           tuple(tuple(r) for t in sched["runs_per_tile"] for r in t))
    if key in _cache:
        return _cache[key]
    from runner import make_runner
    nc = kbuild.build_nc(sched, R=R, repeat=repeat)
    nc.compile()
    run = make_runner(nc, 8)
    _cache[key] = run
    return run


def kernel(**inputs):
    sched, arrs, in_maps = _prep(inputs)
    run = _get_runner(sched)
    results, wall, _ = run(in_maps, K=1)

    S = sched["S"]
    y = np.zeros((50000, 64), np.float32)
    for c in range(8):
        yc = results[c]["y"]
        nos = arrs["node_of_slot"][c]
        for s in range(2):
            realm = nos[s] >= 0
            y[nos[s][realm]] = yc[s * S:(s + 1) * S][realm]
    return y
